# revision 6
# baseline (speedup 1.0000x reference)
"""BREWA (bit-witness) attention on 8 TRN2 NeuronCores.

Sharding: core c = (batch b, head-group g) with b = c // 2, g = c % 2.
Each core computes its batch's attention for 8 of the 16 heads plus the
partial output projection over those heads' Wo columns; the host sums the
two partial projections per batch (the "all-reduce" is 2-way, done on CPU).

Per-core dataflow (all matmuls bf16, fp32 PSUM accumulate):
  xT[b] (host-transposed, bf16)  --PE-->  QT,KT [512,2048] (dims on partitions)
                                 --PE-->  V    [2048,520]  (seq on partitions,
                                                            65 cols/head: 64 V dims + ones)
  QT,KT --PE (K=64, row+col tiled)--> enc psum --ACT tanh--> q_encT,k_encT
        [128, 2048] tiles: 4 heads x 32 bits on partitions, seq on free dim
  per (head-quad, q-tile 512, k-tile 128):
    ST[k,q] via 4 row-tiled K=32 matmuls -> st psum [128, 2048] (head r at 512r)
    exp(ST/sqrt32) on ACT -> SBUF bf16   (softmax w/o max-sub: |scores|<=5.66)
    att[r] += V_aug[kt,h].T @ expST      (psum [65,512]; row 64 = sum_k exp = Z)
  normalize: DVE reciprocal(Z) -> GPSIMD partition_broadcast -> DVE mul -> c_T
  y = c_T.T @ WoT_g  (per-core partial, fp32 out)
"""

import numpy as np
import ml_dtypes

import concourse.bacc as bacc
import concourse.bass as bass
import concourse.mybir as mybir
import concourse.tile as tile
from concourse.bass_utils import run_bass_kernel_spmd

B, N, D = 4, 2048, 1024
H, HD, MB = 16, 64, 32
NCORES = 8
HPG = 8              # heads per group (per core)
GD = HPG * HD        # 512 head dims per group
SCALE = float(1.0 / np.sqrt(MB))

bf16 = mybir.dt.bfloat16
f32 = mybir.dt.float32
BF = ml_dtypes.bfloat16
AF = mybir.ActivationFunctionType

KT_X = D // 128      # 8 contraction tiles over d_model
NT = N // 512        # 4 column tiles of 512 over sequence
MT_QK = GD // 128    # 4 partition tiles of QT/KT
NT128 = N // 128     # 16 row tiles of 128 over sequence
KT_C = GD // 128     # 4 contraction tiles over group head dims

TRACE = False        # set by test.py for profiling runs
TRACE_KW = {}
LAST_RESULTS = None


def build():
    nc = bacc.Bacc("TRN2", target_bir_lowering=False, debug=False,
                   num_devices=NCORES)
    xt = nc.dram_tensor("xt", [D, N], bf16, kind="ExternalInput").ap()
    wq = nc.dram_tensor("wq", [D, GD], bf16, kind="ExternalInput").ap()
    wk = nc.dram_tensor("wk", [D, GD], bf16, kind="ExternalInput").ap()
    wv = nc.dram_tensor("wv", [D, GD], bf16, kind="ExternalInput").ap()
    wenc = nc.dram_tensor("wenc", [128, HPG * MB], bf16, kind="ExternalInput").ap()
    wo = nc.dram_tensor("wo", [GD, D], bf16, kind="ExternalInput").ap()
    y = nc.dram_tensor("y", [N, D], f32, kind="ExternalOutput").ap()

    with tile.TileContext(nc) as tc:
        with (
            tc.tile_pool(name="xtp", bufs=KT_X) as xt_pool,
            tc.tile_pool(name="wp", bufs=3 * KT_X) as w_pool,
            tc.tile_pool(name="wop", bufs=KT_C) as wo_pool,
            tc.tile_pool(name="wencp", bufs=1) as wenc_pool,
            tc.tile_pool(name="qkp", bufs=2 * MT_QK) as qk_pool,
            tc.tile_pool(name="encp", bufs=4) as enc_pool,
            tc.tile_pool(name="vp", bufs=NT128) as v_pool,
            tc.tile_pool(name="expp", bufs=3) as exp_pool,
            tc.tile_pool(name="ctp", bufs=KT_C) as ct_pool,
            tc.tile_pool(name="smallp", bufs=8) as small_pool,
            tc.tile_pool(name="yp", bufs=3) as y_pool,
            tc.tile_pool(name="stp", bufs=1, space="PSUM") as st_pool,
            tc.tile_pool(name="bankp", bufs=4, space="PSUM") as bank_pool,
        ):
            # ---- input loads -------------------------------------------------
            xt_sb = []
            for k in range(KT_X):
                t = xt_pool.tile([128, N], bf16, tag="xt")
                nc.sync.dma_start(t[:], xt[128 * k:128 * (k + 1), :])
                xt_sb.append(t)

            def load_w(w_ap):
                tiles = []
                for k in range(KT_X):
                    t = w_pool.tile([128, GD], bf16, tag="w")
                    nc.sync.dma_start(t[:], w_ap[128 * k:128 * (k + 1), :])
                    tiles.append(t)
                return tiles

            wq_sb = load_w(wq)
            wk_sb = load_w(wk)
            wv_sb = load_w(wv)
            wo_sb = []
            for k in range(KT_C):
                t = wo_pool.tile([128, D], bf16, tag="wo")
                nc.sync.dma_start(t[:], wo[128 * k:128 * (k + 1), :])
                wo_sb.append(t)
            wenc_sb = wenc_pool.tile([128, HPG * MB], bf16, tag="wenc")
            nc.sync.dma_start(wenc_sb[:], wenc[:, :])

            # ---- QT / KT: [512 dims, 2048 seq], dims on partitions ----------
            qT_sb, kT_sb = [], []
            for wsb, dst in ((wq_sb, qT_sb), (wk_sb, kT_sb)):
                for mt in range(MT_QK):
                    t = qk_pool.tile([128, N], bf16, tag="qk")
                    for nt in range(NT):
                        ps = bank_pool.tile([128, 512], f32, tag="bank")
                        for k in range(KT_X):
                            nc.tensor.matmul(
                                ps[:],
                                wsb[k][:, 128 * mt:128 * (mt + 1)],
                                xt_sb[k][:, 512 * nt:512 * (nt + 1)],
                                start=(k == 0), stop=(k == KT_X - 1),
                                skip_group_check=True,
                            )
                        nc.vector.tensor_copy(t[:, 512 * nt:512 * (nt + 1)], ps[:])
                    dst.append(t)

            # ---- V in [seq, dims] with a ones column per head ---------------
            v_sb = []
            for nt in range(NT128):
                t = v_pool.tile([128, HPG * 65], bf16, tag="v")
                ps = bank_pool.tile([128, 512], f32, tag="bank")
                for k in range(KT_X):
                    nc.tensor.matmul(
                        ps[:],
                        xt_sb[k][:, 128 * nt:128 * (nt + 1)],
                        wv_sb[k][:],
                        start=(k == 0), stop=(k == KT_X - 1),
                        skip_group_check=True,
                    )
                vv = t[:, :].rearrange("p (h s) -> p h s", h=HPG)
                nc.vector.tensor_copy(
                    vv[:, :, 0:64],
                    ps[:, :].rearrange("p (h s) -> p h s", h=HPG),
                )
                nc.vector.memset(vv[:, :, 64:65], 1.0)
                v_sb.append(t)

            # ---- witness encoders: tanh(W_enc_h.T @ {Q,K}h.T) ---------------
            # enc tiles: [128, 2048] = 4 heads x 32 bits on partitions
            q_enc, k_enc = [], []
            for src, dst in ((qT_sb, q_enc), (kT_sb, k_enc)):
                for qd in range(2):          # head quad within the group
                    eps = st_pool.tile([128, N], f32, tag="st")
                    for r in range(4):
                        h = 4 * qd + r
                        e = 64 * (h % 2)
                        for nt in range(NT):
                            nc.tensor.matmul(
                                eps[32 * r:32 * (r + 1), 512 * nt:512 * (nt + 1)],
                                wenc_sb[e:e + 64, MB * h:MB * (h + 1)],
                                src[h // 2][e:e + 64, 512 * nt:512 * (nt + 1)],
                                start=True, stop=True,
                                tile_position=(e, 32 * r),
                                skip_group_check=True,
                            )
                    et = enc_pool.tile([128, N], bf16, tag="enc")
                    nc.scalar.activation(et[:], eps[:], AF.Tanh)
                    dst.append(et)

            # ---- c_T accumulator tiles: [512 head dims, 2048 seq] -----------
            ct_sb = [ct_pool.tile([128, N], bf16, tag="ct", name=f"ct{i}")
                     for i in range(KT_C)]

            # ---- attention: ST -> exp -> att.V, per (quad, q-tile) ----------
            for qd in range(2):
                for qt in range(NT):
                    att = [bank_pool.tile([65, 512], f32, tag="bank",
                                          name=f"att{qd}_{qt}_{r}")
                           for r in range(4)]
                    for kt in range(NT128):
                        st = st_pool.tile([128, N], f32, tag="st")
                        for r in range(4):
                            nc.tensor.matmul(
                                st[:, 512 * r:512 * (r + 1)],
                                k_enc[qd][32 * r:32 * (r + 1), 128 * kt:128 * (kt + 1)],
                                q_enc[qd][32 * r:32 * (r + 1), 512 * qt:512 * (qt + 1)],
                                start=True, stop=True,
                                tile_position=(32 * r, 0),
                                skip_group_check=True,
                            )
                        ex = exp_pool.tile([128, N], bf16, tag="exp")
                        nc.scalar.activation(ex[:], st[:], AF.Exp, scale=SCALE)
                        for r in range(4):
                            h = 4 * qd + r
                            nc.tensor.matmul(
                                att[r][:],
                                v_sb[kt][:, 65 * h:65 * h + 65],
                                ex[:, 512 * r:512 * (r + 1)],
                                start=(kt == 0), stop=(kt == NT128 - 1),
                                skip_group_check=True,
                            )
                    for r in range(4):
                        h = 4 * qd + r
                        recip = small_pool.tile([1, 512], f32, tag="recip")
                        nc.vector.reciprocal(recip[:], att[r][64:65, :])
                        bc = small_pool.tile([64, 512], f32, tag="bc")
                        nc.gpsimd.partition_broadcast(bc[:], recip[:])
                        u = 64 * (h % 2)
                        if u == 0:
                            nc.vector.tensor_mul(
                                ct_sb[h // 2][0:64, 512 * qt:512 * (qt + 1)],
                                att[r][0:64, :], bc[:])
                        else:
                            tmp = small_pool.tile([64, 512], bf16, tag="tmp")
                            nc.vector.tensor_mul(tmp[:], att[r][0:64, :], bc[:])
                            nc.sync.dma_start(
                                ct_sb[h // 2][64:128, 512 * qt:512 * (qt + 1)],
                                tmp[:])

            # ---- output projection: y = c_T.T @ WoT_g -----------------------
            for mt in range(NT128):
                for nt2 in range(2):
                    ps = bank_pool.tile([128, 512], f32, tag="bank")
                    for k in range(KT_C):
                        nc.tensor.matmul(
                            ps[:],
                            ct_sb[k][:, 128 * mt:128 * (mt + 1)],
                            wo_sb[k][:, 512 * nt2:512 * (nt2 + 1)],
                            start=(k == 0), stop=(k == KT_C - 1),
                            skip_group_check=True,
                        )
                    yt = y_pool.tile([128, 512], f32, tag="y")
                    nc.vector.tensor_copy(yt[:], ps[:])
                    nc.sync.dma_start(
                        y[128 * mt:128 * (mt + 1), 512 * nt2:512 * (nt2 + 1)],
                        yt[:])
    nc.finalize()
    return nc


_nc_cache = None


def kernel(**inputs):
    global _nc_cache, LAST_RESULTS
    x = np.asarray(inputs["x"], dtype=np.float32)
    Wq = np.asarray(inputs["Wq"], dtype=np.float32)
    Wk = np.asarray(inputs["Wk"], dtype=np.float32)
    Wv = np.asarray(inputs["Wv"], dtype=np.float32)
    We = np.asarray(inputs["W_enc"], dtype=np.float32)
    Wo = np.asarray(inputs["Wo"], dtype=np.float32)

    if _nc_cache is None:
        _nc_cache = build()
    nc = _nc_cache

    xts = [np.ascontiguousarray(x[b].T).astype(BF) for b in range(B)]
    in_maps = []
    for c in range(NCORES):
        b, g = divmod(c, 2)
        gs = g * GD
        we_g = We[g * HPG:(g + 1) * HPG]          # [8, 64, 32]
        we_blk = np.ascontiguousarray(
            we_g.transpose(1, 0, 2).reshape(HD, HPG * MB))  # [64, 256]
        in_maps.append({
            "xt": xts[b],
            "wq": np.ascontiguousarray(Wq[gs:gs + GD, :].T).astype(BF),
            "wk": np.ascontiguousarray(Wk[gs:gs + GD, :].T).astype(BF),
            "wv": np.ascontiguousarray(Wv[gs:gs + GD, :].T).astype(BF),
            "wenc": np.concatenate([we_blk, we_blk], axis=0).astype(BF),
            "wo": np.ascontiguousarray(Wo[:, gs:gs + GD].T).astype(BF),
        })

    res = run_bass_kernel_spmd(
        nc, in_maps, core_ids=list(range(NCORES)),
        trace=TRACE, **TRACE_KW)
    LAST_RESULTS = res

    out = np.empty((B, N, D), dtype=np.float32)
    for b in range(B):
        out[b] = res.results[2 * b]["y"] + res.results[2 * b + 1]["y"]
    return out
